# revision 29
# baseline (speedup 1.0000x reference)
"""ChartingAE Trainium2 kernel — expert-parallel across 8 NeuronCores.

Sharding: one chart per core (C=8 == n_cores). Each core holds its chart's
encoder/decoder weights resident in SBUF and streams all B=4096 rows through
them in feature-major layout ([feature, batch]), so every layer's matmul
contraction dim is the partition dim with no transposes between layers.

Per-core outputs: raw (unweighted) reconstruction xr^T, latent z^T, per-chart
recon error, and the sparsemax chart probabilities p (computed redundantly on
every core; core 0's copy is used). The p-weighted combine across charts — the
unshard step for expert-parallel — plus the tiny [B,C] loss reductions run on
the host in fp64.

Matmul dtype per layer is configurable: fp16 (1 PE pass, ~2.4e-4 rel) or
fp32r (2 PE passes, ~1.2e-4 rel). ELU is computed branch-free as
  elu(u) = min(exp(u) - 1, max(u, 0))
(for u<=0 the min picks exp(u)-1<=0; for u>0 it picks u since exp(u)-1>u),
costing 1 ACT + 2 DVE ops per tile.
"""

import numpy as np

B, D, LDIM, H, C = 4096, 512, 64, 1024, 8
NCORES = 8
NTILE = 512            # batch tile (moving free dim)
NB = B // NTILE        # 8 batch tiles

# per-layer matmul dtype: "fp16" or "fp32r"
CFG = {
    "l1": "fp16", "l2": "fp16", "l3": "fp16", "l4": "fp16",
    "d1": "fp16", "d2": "fp16", "d3": "fp16", "d4": "fp16",
    "logits": "fp16",
}

LAYERS = [
    # name, in_feat, out_feat, elu
    ("l1", D, H, True),
    ("l2", H, H, True),
    ("l3", H, H, True),
    ("l4", H, LDIM, False),   # -> z
    ("d1", LDIM, H, True),
    ("d2", H, H, True),
    ("d3", H, H, True),
    ("d4", H, D, False),      # -> xr
]

_compiled = {}


def _round_fp32r(a: np.ndarray) -> np.ndarray:
    u = np.ascontiguousarray(a, dtype=np.float32).view(np.uint32).astype(np.uint64)
    lsb = (u >> 12) & 1
    u = (((u + 0x7FF + lsb) >> 12) << 12).astype(np.uint32)
    return u.view(np.float32)


def _prep(a: np.ndarray, kind: str) -> np.ndarray:
    if kind == "fp16":
        return np.ascontiguousarray(a, dtype=np.float16)
    return _round_fp32r(np.ascontiguousarray(a, dtype=np.float32))


def _build():
    """Build + compile the SPMD bass program (identical on all 8 cores)."""
    import concourse.bacc as bacc
    import concourse.tile as tile
    import concourse.mybir as mybir

    dt = mybir.dt
    AF = mybir.ActivationFunctionType
    ALU = mybir.AluOpType

    def bdt(kind):
        return dt.float16 if kind == "fp16" else dt.float32r

    nc = bacc.Bacc("TRN2", target_bir_lowering=False, debug=False)

    # ---- dram parameters ----
    xa = nc.declare_dram_parameter("xa", [D, B], bdt(CFG["l1"]), isOutput=False)
    wps = {}
    bps = {}
    for name, fin, fout, _ in LAYERS:
        wps[name] = nc.declare_dram_parameter(f"w_{name}", [fin, fout], bdt(CFG[name]), isOutput=False)
        bps[name] = nc.declare_dram_parameter(f"b_{name}", [fout], dt.float32, isOutput=False)
    wp = nc.declare_dram_parameter("wp", [D, C], bdt(CFG["logits"]), isOutput=False)
    bp_rep = nc.declare_dram_parameter("bp_rep", [128, C], dt.float32, isOutput=False)

    xrT = nc.declare_dram_parameter("xrT", [D, B], dt.float32, isOutput=True)
    zT = nc.declare_dram_parameter("zT", [LDIM, B], dt.float32, isOutput=True)
    err = nc.declare_dram_parameter("err", [1, B], dt.float32, isOutput=True)
    p_out = nc.declare_dram_parameter("p_out", [128, B * C // 128], dt.float32, isOutput=True)

    with tile.TileContext(nc) as tc:
        import contextlib
        with contextlib.ExitStack() as ctx:
            wpool = ctx.enter_context(tc.tile_pool(name="weights", bufs=1))
            cpool = ctx.enter_context(tc.tile_pool(name="consts", bufs=1))
            xpool = ctx.enter_context(tc.tile_pool(name="x", bufs=3))
            h2b = ctx.enter_context(tc.tile_pool(name="h2b", bufs=36))
            h4b = ctx.enter_context(tc.tile_pool(name="h4b", bufs=12))
            tpool = ctx.enter_context(tc.tile_pool(name="temps", bufs=4))
            opool = ctx.enter_context(tc.tile_pool(name="outs", bufs=6))
            spool = ctx.enter_context(tc.tile_pool(name="small", bufs=3))
            ppool = ctx.enter_context(tc.tile_pool(name="pwork", bufs=1))
            ps = ctx.enter_context(tc.tile_pool(name="ps", bufs=6, space="PSUM"))
            ps1 = ctx.enter_context(tc.tile_pool(name="ps1", bufs=1, space="PSUM"))
            ps2 = ctx.enter_context(tc.tile_pool(name="ps2", bufs=1, space="PSUM"))

            # ---- weights: l1 + logits first so batch-0 compute starts ASAP;
            # the rest are DMA'd after batch-0 L1 is emitted ----
            wt = {}   # name -> list of k-tiles [128, fout]
            bt = {}   # name -> bias tile [128, mtiles]

            def load_w(name):
                fin, fout = dict((n, (fi, fo)) for n, fi, fo, _ in LAYERS)[name]
                kt = (fin + 127) // 128
                tiles = []
                for k in range(kt):
                    kp = min(128, fin - k * 128)
                    t = wpool.tile([kp, fout], bdt(CFG[name]), tag=f"w_{name}_{k}")
                    nc.sync.dma_start(t[:], wps[name][k * 128:k * 128 + kp, :])
                    tiles.append(t)
                wt[name] = tiles

            for name, fin, fout, _ in LAYERS:
                mt = (fout + 127) // 128
                bb = cpool.tile([min(128, fout), mt], dt.float32, tag=f"b_{name}")
                nc.sync.dma_start(
                    bb[:], bps[name][:].rearrange("(m p) -> p m", m=mt))
                bt[name] = bb
            load_w("l1")
            wpt = []
            for k in range(D // 128):
                t = wpool.tile([128, C], bdt(CFG["logits"]), tag=f"wp_{k}")
                nc.sync.dma_start(t[:], wp[k * 128:(k + 1) * 128, :])
                wpt.append(t)
            bpr = cpool.tile([128, C], dt.float32, tag="bp")
            nc.sync.dma_start(bpr[:], bp_rep[:, :])
            ones16 = cpool.tile([128, 1], dt.float16, tag="ones")
            nc.vector.memset(ones16[:], 1.0)
            # sparsemax workspace [128, NB*4*C]
            Lp = ppool.tile([128, B * C // 128], dt.float32, tag="Lp")

            act_dt = bdt(CFG["l1"])

            def act_pool(kind):
                return h2b if kind == "fp16" else h4b

            # ---- software-pipelined emission: Tile schedules each engine in
            # program order, so batch-tile t+1's matmuls can only fill tile
            # t's layer-boundary bubbles if we interleave them statically.
            # Chunk (t, c) = layer c of batch-tile t, emitted by priority
            # t*OFFS + c so two tiles are in flight, offset by OFFS layers.
            OFFS = 4
            state = {}   # nb -> dict(xk=..., cur=..., logit_s=...)

            def emit_chunk(nb, li):
                bs = nb * NTILE
                name, fin, fout, elu = LAYERS[li]
                if li == 0:
                    xk = []
                    for k in range(D // 128):
                        t = xpool.tile([128, NTILE], act_dt, tag=f"x{k}")
                        nc.sync.dma_start(t[:], xa[k * 128:(k + 1) * 128, bs:bs + NTILE])
                        xk.append(t)
                    state[nb] = {"xk": xk, "cur": xk}
                st = state[nb]
                cur = st["cur"]
                xk = st["xk"]
                kt = (fin + 127) // 128
                mt = (fout + 127) // 128
                nxt = []
                for m in range(mt):
                    mp = min(128, fout - m * 128)
                    pt = ps.tile([mp, NTILE], dt.float32, tag="ps")
                    for k in range(kt):
                        nc.tensor.matmul(
                            pt[:], wt[name][k][:, m * 128:m * 128 + mp], cur[k][:],
                            start=(k == 0), stop=(k == kt - 1))
                    bcol = bt[name][0:mp, m:m + 1]
                    if elu:
                        # elu(u) = min(exp(u)-1, max(u,0)), u = y + b
                        # engine split keeps ACT/DVE/GpSimd all below PE
                        E = tpool.tile([mp, NTILE], dt.float32, tag="E")
                        nc.scalar.activation(E[:], pt[:], AF.Exp, bias=bcol)
                        r = tpool.tile([mp, NTILE], dt.float32, tag="r")
                        if m % 8 < 3:
                            nc.scalar.activation(r[:], pt[:], AF.Relu, bias=bcol)
                        else:
                            nc.vector.tensor_scalar(r[:], pt[:], bcol, 0.0, ALU.add, ALU.max)
                        hname = "h2" if CFG_next_dtype(name) == "fp16" else "h4"
                        hout = act_pool(CFG_next_dtype(name)).tile(
                            [mp, NTILE], bdt(CFG_next_dtype(name)), tag=hname)
                        nc.vector.scalar_tensor_tensor(
                            hout[:], E[:], 1.0, r[:], ALU.subtract, ALU.min)
                        nxt.append(hout)
                    elif name == "l4":
                        # z: export fp32 + feed d1
                        z_s = opool.tile([mp, NTILE], dt.float32, tag="z_s")
                        nc.scalar.activation(z_s[:], pt[:], AF.Identity, bias=bcol)
                        nc.sync.dma_start(zT[:, bs:bs + NTILE], z_s[:])
                        zr = act_pool(CFG_next_dtype(name)).tile(
                            [mp, NTILE], bdt(CFG_next_dtype(name)),
                            tag="h2" if CFG_next_dtype(name) == "fp16" else "h4")
                        nc.vector.tensor_scalar(zr[:], pt[:], bcol, None, ALU.add)
                        nxt.append(zr)
                    else:
                        # d4: xr m-tile -> export + error
                        xr = opool.tile([mp, NTILE], dt.float32, tag="xr")
                        nc.scalar.activation(xr[:], pt[:], AF.Identity, bias=bcol)
                        nc.sync.dma_start(
                            xrT[m * 128:m * 128 + mp, bs:bs + NTILE], xr[:])
                        diff = tpool.tile([mp, NTILE], dt.float32, tag="diff")
                        nc.gpsimd.tensor_tensor(diff[:], xr[:], xk[m][:], ALU.subtract)
                        sq = tpool.tile([mp, NTILE], dt.float16, tag="sq")
                        nc.gpsimd.tensor_tensor(sq[:], diff[:], diff[:], ALU.mult)
                        eps = ps1.tile([1, NTILE], dt.float32, tag="eps")
                        nc.tensor.matmul(eps[:], ones16[:], sq[:],
                                         start=(m == 0), stop=(m == mt - 1))
                        if m == mt - 1:
                            err_s = spool.tile([1, NTILE], dt.float32, tag="err_s")
                            nc.vector.tensor_copy(err_s[:], eps[:])
                            nc.sync.dma_start(err[:, bs:bs + NTILE], err_s[:])
                        nxt.append(xr)
                st["cur"] = nxt
                if nb == 0 and name == "l1":
                    for nm in ("l2", "l3", "l4", "d1", "d2", "d3", "d4"):
                        load_w(nm)
                if name == "l4":
                    # chart logits, computed directly batch-major:
                    # logitsT[b_lo, c] = sum_k x[k, b].T @ wp[k, c] — x slice
                    # is the stationary operand, so no transposes are needed.
                    # Emitted here as PE filler between L4 and D4(t-1)/D1.
                    for j in range(NTILE // 128):
                        lt = ps2.tile([128, C], dt.float32, tag="lps")
                        for k in range(D // 128):
                            nc.tensor.matmul(
                                lt[:], xk[k][:, j * 128:(j + 1) * 128], wpt[k][:],
                                start=(k == 0), stop=(k == D // 128 - 1))
                        col = (nb * (NTILE // 128) + j) * C
                        nc.vector.tensor_copy(Lp[:, col:col + C], lt[:])

            # decoder chunks sit at half-priorities so tile t's decoder
            # interleaves one-to-one with tile t+1's encoder — every serial
            # dependency (z->D1, d_i ELU -> D_{i+1}) gets a full chunk of
            # independent PE work emitted in between.
            PRIO = [0, 1, 2, 3, 4.5, 5.5, 6.5, 7.5]
            sched = sorted((t * OFFS + PRIO[c], t, c)
                           for t in range(NB) for c in range(len(LAYERS)))
            for _, t_, c_ in sched:
                emit_chunk(t_, c_)

            # ---- sparsemax on Lp [128, NB*4, C] ----
            F = B * C // 128
            G = F // C          # groups of C along free dim
            z3 = Lp[:].rearrange("p (g c) -> p g c", c=C)
            nc.vector.tensor_tensor(
                z3, z3, bpr[:, None, :].broadcast_to([128, G, C]), ALU.add)
            cnt = ppool.tile([128, F], dt.float32, tag="cnt")
            nc.vector.memset(cnt[:], 1.0)
            Tt = ppool.tile([128, F], dt.float32, tag="Tt")
            nc.vector.tensor_copy(Tt[:], Lp[:])
            c3 = cnt[:].rearrange("p (g c) -> p g c", c=C)
            T3 = Tt[:].rearrange("p (g c) -> p g c", c=C)
            ge = ppool.tile([128, F], dt.float32, tag="ge")
            gz = ppool.tile([128, F], dt.float32, tag="gz")
            ge3 = ge[:].rearrange("p (g c) -> p g c", c=C)
            gz3 = gz[:].rearrange("p (g c) -> p g c", c=C)
            for s in range(1, C):
                w = C - s
                # region 1: i in [0,w) vs j=i+s ; region 2 (wrap): i in [w,C) vs j=i-w
                nc.vector.tensor_tensor(ge3[:, :, 0:w], z3[:, :, s:C], z3[:, :, 0:w], ALU.is_ge)
                nc.vector.tensor_tensor(ge3[:, :, w:C], z3[:, :, 0:s], z3[:, :, w:C], ALU.is_ge)
                nc.vector.tensor_tensor(gz3[:, :, 0:w], ge3[:, :, 0:w], z3[:, :, s:C], ALU.mult)
                nc.vector.tensor_tensor(gz3[:, :, w:C], ge3[:, :, w:C], z3[:, :, 0:s], ALU.mult)
                nc.vector.tensor_tensor(cnt[:], cnt[:], ge[:], ALU.add)
                nc.vector.tensor_tensor(Tt[:], Tt[:], gz[:], ALU.add)
            # in_sup = (1 + cnt*z) > T
            u = ppool.tile([128, F], dt.float32, tag="u")
            nc.vector.tensor_tensor(u[:], cnt[:], Lp[:], ALU.mult)
            nc.vector.tensor_scalar(u[:], u[:], 1.0, None, ALU.add)
            sup = ppool.tile([128, F], dt.float32, tag="sup")
            nc.vector.tensor_tensor(sup[:], u[:], Tt[:], ALU.is_gt)
            sup3 = sup[:].rearrange("p (g c) -> p g c", c=C)
            kz = ppool.tile([128, G], dt.float32, tag="kz")
            nc.vector.tensor_reduce(kz[:], sup3, mybir.AxisListType.X, ALU.add)
            zs = ppool.tile([128, F], dt.float32, tag="zs")
            nc.vector.tensor_tensor(zs[:], Lp[:], sup[:], ALU.mult)
            Ss = ppool.tile([128, G], dt.float32, tag="Ss")
            nc.vector.tensor_reduce(Ss[:], zs[:].rearrange("p (g c) -> p g c", c=C), mybir.AxisListType.X, ALU.add)
            nc.vector.tensor_scalar(Ss[:], Ss[:], 1.0, None, ALU.subtract)
            rk = ppool.tile([128, G], dt.float32, tag="rk")
            nc.vector.reciprocal(rk[:], kz[:])
            tau = ppool.tile([128, G], dt.float32, tag="tau")
            nc.vector.tensor_tensor(tau[:], Ss[:], rk[:], ALU.mult)
            pfin = ppool.tile([128, F], dt.float32, tag="pfin")
            tau_b = tau[:, :, None].broadcast_to([128, G, C])
            nc.vector.tensor_tensor(pfin[:].rearrange("p (g c) -> p g c", c=C), z3, tau_b, ALU.subtract)
            nc.vector.tensor_scalar(pfin[:], pfin[:], 0.0, None, ALU.max)
            nc.sync.dma_start(p_out[:, :], pfin[:])

    nc.compile()
    return nc


def CFG_next_dtype(name):
    nxt = {"l1": "l2", "l2": "l3", "l3": "l4", "l4": "d1",
           "d1": "d2", "d2": "d3", "d3": "d4", "d4": None}[name]
    return CFG[nxt] if nxt else "fp32"


def kernel(x, We1, be1, We2, be2, We3, be3, We4, be4,
           Wd1, bd1, Wd2, bd2, Wd3, bd3, Wd4, bd4, Wp, bp):
    from concourse.bass_utils import run_bass_kernel_spmd

    if "nc" not in _compiled:
        _compiled["nc"] = _build()
    nc = _compiled["nc"]

    x = np.asarray(x, dtype=np.float32)
    xT = np.ascontiguousarray(x.T)
    xa = _prep(xT, CFG["l1"])
    wpa = _prep(np.asarray(Wp, np.float32), CFG["logits"])
    bp_rep = np.tile(np.asarray(bp, np.float32)[None, :], (128, 1))

    Ws = {"l1": We1, "l2": We2, "l3": We3, "l4": We4,
          "d1": Wd1, "d2": Wd2, "d3": Wd3, "d4": Wd4}
    Bs = {"l1": be1, "l2": be2, "l3": be3, "l4": be4,
          "d1": bd1, "d2": bd2, "d3": bd3, "d4": bd4}

    in_maps = []
    for c in range(NCORES):
        m = {"xa": xa, "wp": wpa, "bp_rep": bp_rep}
        for name in Ws:
            m[f"w_{name}"] = _prep(np.asarray(Ws[name][c], np.float32), CFG[name])
            m[f"b_{name}"] = np.ascontiguousarray(Bs[name][c], np.float32)
        in_maps.append(m)

    res = run_bass_kernel_spmd(nc, in_maps, list(range(NCORES)))
    results = res.results

    # ---- host-side unshard + combine ----
    xrT_all = np.stack([results[c]["xrT"] for c in range(NCORES)])   # [C, D, B]
    zT_all = np.stack([results[c]["zT"] for c in range(NCORES)])     # [C, L, B]
    errs = np.stack([results[c]["err"][0] for c in range(NCORES)], axis=1)  # [B, C]
    packed = results[0]["p_out"]                                     # [128, B*C/128]
    p = packed.reshape(128, B // 128, C).transpose(1, 0, 2).reshape(B, C)

    z = np.ascontiguousarray(zT_all.transpose(2, 0, 1))              # [B, C, L]

    pd = p.astype(np.float64)
    recon_x = np.zeros((B, D), dtype=np.float64)
    for c in range(NCORES):
        recon_x += xrT_all[c].T.astype(np.float64) * pd[:, c:c + 1]

    errs64 = errs.astype(np.float64)
    recon_loss = (pd * errs64).sum(axis=-1).mean()
    # softmax(-errs) computed stably:
    m_ = (-errs64).max(axis=-1, keepdims=True)
    q = np.exp(-errs64 - m_)
    q /= q.sum(axis=-1, keepdims=True)
    trans_loss = -(q * np.log(pd + 1e-8)).sum(axis=-1).mean()
    mean_p = pd.mean(axis=0)
    nondom_loss = ((mean_p - 1.0 / C) ** 2).sum()
    total_loss = recon_loss + trans_loss + nondom_loss
    xd = x.astype(np.float64)
    mse_loss = ((recon_x - xd) ** 2).mean()

    return (recon_x.astype(np.float32),
            z.astype(np.float32),
            p.astype(np.float32),
            np.float32(total_loss),
            np.float32(recon_loss),
            np.float32(nondom_loss),
            np.float32(trans_loss),
            np.float32(mse_loss))


# revision 33
# speedup vs baseline: 1.0044x; 1.0044x over previous
"""ChartingAE Trainium2 kernel — expert-parallel across 8 NeuronCores.

Sharding: one chart per core (C=8 == n_cores). Each core holds its chart's
encoder/decoder weights resident in SBUF and streams all B=4096 rows through
them in feature-major layout ([feature, batch]), so every layer's matmul
contraction dim is the partition dim with no transposes between layers.

Per-core outputs: raw (unweighted) reconstruction xr^T, latent z^T, per-chart
recon error, and the sparsemax chart probabilities p (computed redundantly on
every core; core 0's copy is used). The p-weighted combine across charts — the
unshard step for expert-parallel — plus the tiny [B,C] loss reductions run on
the host in fp64.

Matmul dtype per layer is configurable: fp16 (1 PE pass, ~2.4e-4 rel) or
fp32r (2 PE passes, ~1.2e-4 rel). ELU is computed branch-free as
  elu(u) = min(exp(u) - 1, max(u, 0))
(for u<=0 the min picks exp(u)-1<=0; for u>0 it picks u since exp(u)-1>u),
costing 1 ACT + 2 DVE ops per tile.
"""

import numpy as np

B, D, LDIM, H, C = 4096, 512, 64, 1024, 8
NCORES = 8
NTILE = 512            # batch tile (moving free dim)
NB = B // NTILE        # 8 batch tiles

# per-layer matmul dtype: "fp16" or "fp32r"
CFG = {
    "l1": "fp16", "l2": "fp16", "l3": "fp16", "l4": "fp16",
    "d1": "fp16", "d2": "fp16", "d3": "fp16", "d4": "fp16",
    "logits": "fp16",
}

LAYERS = [
    # name, in_feat, out_feat, elu
    ("l1", D, H, True),
    ("l2", H, H, True),
    ("l3", H, H, True),
    ("l4", H, LDIM, False),   # -> z
    ("d1", LDIM, H, True),
    ("d2", H, H, True),
    ("d3", H, H, True),
    ("d4", H, D, False),      # -> xr
]

_compiled = {}


def _round_fp32r(a: np.ndarray) -> np.ndarray:
    u = np.ascontiguousarray(a, dtype=np.float32).view(np.uint32).astype(np.uint64)
    lsb = (u >> 12) & 1
    u = (((u + 0x7FF + lsb) >> 12) << 12).astype(np.uint32)
    return u.view(np.float32)


def _prep(a: np.ndarray, kind: str) -> np.ndarray:
    if kind == "fp16":
        return np.ascontiguousarray(a, dtype=np.float16)
    return _round_fp32r(np.ascontiguousarray(a, dtype=np.float32))


def _build():
    """Build + compile the SPMD bass program (identical on all 8 cores)."""
    import concourse.bacc as bacc
    import concourse.tile as tile
    import concourse.mybir as mybir

    dt = mybir.dt
    AF = mybir.ActivationFunctionType
    ALU = mybir.AluOpType

    def bdt(kind):
        return dt.float16 if kind == "fp16" else dt.float32r

    nc = bacc.Bacc("TRN2", target_bir_lowering=False, debug=False)

    # ---- dram parameters ----
    xa = nc.declare_dram_parameter("xa", [D, B], bdt(CFG["l1"]), isOutput=False)
    wps = {}
    bps = {}
    for name, fin, fout, _ in LAYERS:
        wps[name] = nc.declare_dram_parameter(f"w_{name}", [fin, fout], bdt(CFG[name]), isOutput=False)
        bps[name] = nc.declare_dram_parameter(f"b_{name}", [fout], dt.float32, isOutput=False)
    wp = nc.declare_dram_parameter("wp", [D, C], bdt(CFG["logits"]), isOutput=False)
    bp_rep = nc.declare_dram_parameter("bp_rep", [128, C], dt.float32, isOutput=False)

    xrT = nc.declare_dram_parameter("xrT", [D, B], dt.float32, isOutput=True)
    zT = nc.declare_dram_parameter("zT", [LDIM, B], dt.float32, isOutput=True)
    err = nc.declare_dram_parameter("err", [1, B], dt.float32, isOutput=True)
    p_out = nc.declare_dram_parameter("p_out", [128, B * C // 128], dt.float32, isOutput=True)

    with tile.TileContext(nc) as tc:
        import contextlib
        with contextlib.ExitStack() as ctx:
            wpool = ctx.enter_context(tc.tile_pool(name="weights", bufs=1))
            cpool = ctx.enter_context(tc.tile_pool(name="consts", bufs=1))
            xpool = ctx.enter_context(tc.tile_pool(name="x", bufs=3))
            h2b = ctx.enter_context(tc.tile_pool(name="h2b", bufs=36))
            h4b = ctx.enter_context(tc.tile_pool(name="h4b", bufs=12))
            tpool = ctx.enter_context(tc.tile_pool(name="temps", bufs=4))
            opool = ctx.enter_context(tc.tile_pool(name="outs", bufs=6))
            spool = ctx.enter_context(tc.tile_pool(name="small", bufs=3))
            ppool = ctx.enter_context(tc.tile_pool(name="pwork", bufs=1))
            ps = ctx.enter_context(tc.tile_pool(name="ps", bufs=6, space="PSUM"))
            ps1 = ctx.enter_context(tc.tile_pool(name="ps1", bufs=1, space="PSUM"))
            ps2 = ctx.enter_context(tc.tile_pool(name="ps2", bufs=1, space="PSUM"))

            # ---- weights: l1 + logits first so batch-0 compute starts ASAP;
            # the rest are DMA'd after batch-0 L1 is emitted ----
            wt = {}   # name -> list of k-tiles [128, fout]
            bt = {}   # name -> bias tile [128, mtiles]

            def load_w(name):
                fin, fout = dict((n, (fi, fo)) for n, fi, fo, _ in LAYERS)[name]
                kt = (fin + 127) // 128
                tiles = []
                for k in range(kt):
                    kp = min(128, fin - k * 128)
                    t = wpool.tile([kp, fout], bdt(CFG[name]), tag=f"w_{name}_{k}")
                    nc.sync.dma_start(t[:], wps[name][k * 128:k * 128 + kp, :])
                    tiles.append(t)
                wt[name] = tiles

            for name, fin, fout, _ in LAYERS:
                mt = (fout + 127) // 128
                bb = cpool.tile([min(128, fout), mt], dt.float32, tag=f"b_{name}")
                nc.sync.dma_start(
                    bb[:], bps[name][:].rearrange("(m p) -> p m", m=mt))
                bt[name] = bb
            load_w("l1")
            wpt = []
            for k in range(D // 128):
                t = wpool.tile([128, C], bdt(CFG["logits"]), tag=f"wp_{k}")
                nc.sync.dma_start(t[:], wp[k * 128:(k + 1) * 128, :])
                wpt.append(t)
            bpr = cpool.tile([128, C], dt.float32, tag="bp")
            nc.sync.dma_start(bpr[:], bp_rep[:, :])
            ones16 = cpool.tile([128, 1], dt.float16, tag="ones")
            nc.vector.memset(ones16[:], 1.0)
            # sparsemax workspace [128, NB*4*C]
            Lp = ppool.tile([128, B * C // 128], dt.float32, tag="Lp")

            act_dt = bdt(CFG["l1"])

            def act_pool(kind):
                return h2b if kind == "fp16" else h4b

            # ---- software-pipelined emission: Tile schedules each engine in
            # program order, so batch-tile t+1's matmuls can only fill tile
            # t's layer-boundary bubbles if we interleave them statically.
            # Chunk (t, c) = layer c of batch-tile t, emitted by priority
            # t*OFFS + c so two tiles are in flight, offset by OFFS layers.
            OFFS = 4
            state = {}   # nb -> dict(xk=..., cur=...)

            def emit_logits(nb, j):
                # logitsT[b_lo, c] = sum_k x[k, b].T @ wp[k, c] — x slice is
                # the stationary operand, so logits come out batch-major and
                # no transposes are needed
                xkn = state[nb]["xk"]
                lt = ps2.tile([128, C], dt.float32, tag="lps")
                for k in range(D // 128):
                    nc.tensor.matmul(
                        lt[:], xkn[k][:, j * 128:(j + 1) * 128], wpt[k][:],
                        start=(k == 0), stop=(k == D // 128 - 1))
                col = (nb * (NTILE // 128) + j) * C
                nc.vector.tensor_copy(Lp[:, col:col + C], lt[:])

            def emit_chunk(nb, li):
                bs = nb * NTILE
                name, fin, fout, elu = LAYERS[li]
                if li == 0:
                    xk = []
                    for k in range(D // 128):
                        t = xpool.tile([128, NTILE], act_dt, tag=f"x{k}")
                        nc.sync.dma_start(t[:], xa[k * 128:(k + 1) * 128, bs:bs + NTILE])
                        xk.append(t)
                    state[nb] = {"xk": xk, "cur": xk}
                st = state[nb]
                cur = st["cur"]
                xk = st["xk"]
                kt = (fin + 127) // 128
                mt = (fout + 127) // 128
                nxt = []
                for m in range(mt):
                    mp = min(128, fout - m * 128)
                    pt = ps.tile([mp, NTILE], dt.float32, tag="ps")
                    for k in range(kt):
                        nc.tensor.matmul(
                            pt[:], wt[name][k][:, m * 128:m * 128 + mp], cur[k][:],
                            start=(k == 0), stop=(k == kt - 1))
                    bcol = bt[name][0:mp, m:m + 1]
                    if elu:
                        # elu(u) = min(exp(u)-1, max(u,0)), u = y + b
                        # engine split keeps ACT/DVE/GpSimd all below PE
                        E = tpool.tile([mp, NTILE], dt.float32, tag="E")
                        nc.scalar.activation(E[:], pt[:], AF.Exp, bias=bcol)
                        r = tpool.tile([mp, NTILE], dt.float32, tag="r")
                        if m % 8 < 3:
                            nc.scalar.activation(r[:], pt[:], AF.Relu, bias=bcol)
                        else:
                            nc.vector.tensor_scalar(r[:], pt[:], bcol, 0.0, ALU.add, ALU.max)
                        hname = "h2" if CFG_next_dtype(name) == "fp16" else "h4"
                        hout = act_pool(CFG_next_dtype(name)).tile(
                            [mp, NTILE], bdt(CFG_next_dtype(name)), tag=hname)
                        nc.vector.scalar_tensor_tensor(
                            hout[:], E[:], 1.0, r[:], ALU.subtract, ALU.min)
                        nxt.append(hout)
                    elif name == "l4":
                        # z: export fp32 + feed d1
                        z_s = opool.tile([mp, NTILE], dt.float32, tag="z_s")
                        nc.scalar.activation(z_s[:], pt[:], AF.Identity, bias=bcol)
                        nc.sync.dma_start(zT[:, bs:bs + NTILE], z_s[:])
                        zr = act_pool(CFG_next_dtype(name)).tile(
                            [mp, NTILE], bdt(CFG_next_dtype(name)),
                            tag="h2" if CFG_next_dtype(name) == "fp16" else "h4")
                        nc.scalar.activation(zr[:], pt[:], AF.Identity, bias=bcol)
                        nxt.append(zr)
                    else:
                        # d4: xr m-tile -> export + error
                        xr = opool.tile([mp, NTILE], dt.float32, tag="xr")
                        nc.scalar.activation(xr[:], pt[:], AF.Identity, bias=bcol)
                        nc.sync.dma_start(
                            xrT[m * 128:m * 128 + mp, bs:bs + NTILE], xr[:])
                        diff = tpool.tile([mp, NTILE], dt.float32, tag="diff")
                        nc.gpsimd.tensor_tensor(diff[:], xr[:], xk[m][:], ALU.subtract)
                        sq = tpool.tile([mp, NTILE], dt.float16, tag="sq")
                        nc.gpsimd.tensor_tensor(sq[:], diff[:], diff[:], ALU.mult)
                        eps = ps1.tile([1, NTILE], dt.float32, tag="eps")
                        nc.tensor.matmul(eps[:], ones16[:], sq[:],
                                         start=(m == 0), stop=(m == mt - 1))
                        if m == mt - 1:
                            err_s = spool.tile([1, NTILE], dt.float32, tag="err_s")
                            nc.vector.tensor_copy(err_s[:], eps[:])
                            nc.sync.dma_start(err[:, bs:bs + NTILE], err_s[:])
                        nxt.append(xr)
                        # interleave the next tile's logits blocks between the
                        # d4 m-tiles: keeps 213ns streams around the LDW-heavy
                        # little logits matmuls so HAM stays warm
                        if nb + 1 in state:
                            emit_logits(nb + 1, m)
                st["cur"] = nxt
                if nb == 0 and name == "l1":
                    for nm in ("l2", "l3", "l4", "d1", "d2", "d3", "d4"):
                        load_w(nm)
                if name == "l4" and nb == 0:
                    # tile 0 has no predecessor d4 chunk to host its logits
                    for j in range(NTILE // 128):
                        emit_logits(0, j)

            # decoder chunks sit at half-priorities so tile t's decoder
            # interleaves one-to-one with tile t+1's encoder — every serial
            # dependency (z->D1, d_i ELU -> D_{i+1}) gets a full chunk of
            # independent PE work emitted in between.
            PRIO = [0, 1, 2, 3, 4.5, 5.5, 6.5, 7.5]
            sched = sorted((t * OFFS + PRIO[c], t, c)
                           for t in range(NB) for c in range(len(LAYERS)))
            for _, t_, c_ in sched:
                emit_chunk(t_, c_)

            # ---- sparsemax on Lp [128, NB*4, C] ----
            F = B * C // 128
            G = F // C          # groups of C along free dim
            z3 = Lp[:].rearrange("p (g c) -> p g c", c=C)
            nc.vector.tensor_tensor(
                z3, z3, bpr[:, None, :].broadcast_to([128, G, C]), ALU.add)
            cnt = ppool.tile([128, F], dt.float32, tag="cnt")
            nc.vector.memset(cnt[:], 1.0)
            Tt = ppool.tile([128, F], dt.float32, tag="Tt")
            nc.vector.tensor_copy(Tt[:], Lp[:])
            c3 = cnt[:].rearrange("p (g c) -> p g c", c=C)
            T3 = Tt[:].rearrange("p (g c) -> p g c", c=C)
            ge = ppool.tile([128, F], dt.float32, tag="ge")
            gz = ppool.tile([128, F], dt.float32, tag="gz")
            ge3 = ge[:].rearrange("p (g c) -> p g c", c=C)
            gz3 = gz[:].rearrange("p (g c) -> p g c", c=C)
            for s in range(1, C):
                w = C - s
                # region 1: i in [0,w) vs j=i+s ; region 2 (wrap): i in [w,C) vs j=i-w
                nc.vector.tensor_tensor(ge3[:, :, 0:w], z3[:, :, s:C], z3[:, :, 0:w], ALU.is_ge)
                nc.vector.tensor_tensor(ge3[:, :, w:C], z3[:, :, 0:s], z3[:, :, w:C], ALU.is_ge)
                nc.vector.tensor_tensor(gz3[:, :, 0:w], ge3[:, :, 0:w], z3[:, :, s:C], ALU.mult)
                nc.vector.tensor_tensor(gz3[:, :, w:C], ge3[:, :, w:C], z3[:, :, 0:s], ALU.mult)
                nc.vector.tensor_tensor(cnt[:], cnt[:], ge[:], ALU.add)
                nc.vector.tensor_tensor(Tt[:], Tt[:], gz[:], ALU.add)
            # in_sup = (1 + cnt*z) > T
            u = ppool.tile([128, F], dt.float32, tag="u")
            nc.vector.tensor_tensor(u[:], cnt[:], Lp[:], ALU.mult)
            nc.vector.tensor_scalar(u[:], u[:], 1.0, None, ALU.add)
            sup = ppool.tile([128, F], dt.float32, tag="sup")
            nc.vector.tensor_tensor(sup[:], u[:], Tt[:], ALU.is_gt)
            sup3 = sup[:].rearrange("p (g c) -> p g c", c=C)
            kz = ppool.tile([128, G], dt.float32, tag="kz")
            nc.vector.tensor_reduce(kz[:], sup3, mybir.AxisListType.X, ALU.add)
            zs = ppool.tile([128, F], dt.float32, tag="zs")
            nc.vector.tensor_tensor(zs[:], Lp[:], sup[:], ALU.mult)
            Ss = ppool.tile([128, G], dt.float32, tag="Ss")
            nc.vector.tensor_reduce(Ss[:], zs[:].rearrange("p (g c) -> p g c", c=C), mybir.AxisListType.X, ALU.add)
            nc.vector.tensor_scalar(Ss[:], Ss[:], 1.0, None, ALU.subtract)
            rk = ppool.tile([128, G], dt.float32, tag="rk")
            nc.vector.reciprocal(rk[:], kz[:])
            tau = ppool.tile([128, G], dt.float32, tag="tau")
            nc.vector.tensor_tensor(tau[:], Ss[:], rk[:], ALU.mult)
            pfin = ppool.tile([128, F], dt.float32, tag="pfin")
            tau_b = tau[:, :, None].broadcast_to([128, G, C])
            nc.vector.tensor_tensor(pfin[:].rearrange("p (g c) -> p g c", c=C), z3, tau_b, ALU.subtract)
            nc.vector.tensor_scalar(pfin[:], pfin[:], 0.0, None, ALU.max)
            nc.sync.dma_start(p_out[:, :], pfin[:])

    nc.compile()
    return nc


def CFG_next_dtype(name):
    nxt = {"l1": "l2", "l2": "l3", "l3": "l4", "l4": "d1",
           "d1": "d2", "d2": "d3", "d3": "d4", "d4": None}[name]
    return CFG[nxt] if nxt else "fp32"


def kernel(x, We1, be1, We2, be2, We3, be3, We4, be4,
           Wd1, bd1, Wd2, bd2, Wd3, bd3, Wd4, bd4, Wp, bp):
    from concourse.bass_utils import run_bass_kernel_spmd

    if "nc" not in _compiled:
        _compiled["nc"] = _build()
    nc = _compiled["nc"]

    x = np.asarray(x, dtype=np.float32)
    xT = np.ascontiguousarray(x.T)
    xa = _prep(xT, CFG["l1"])
    wpa = _prep(np.asarray(Wp, np.float32), CFG["logits"])
    bp_rep = np.tile(np.asarray(bp, np.float32)[None, :], (128, 1))

    Ws = {"l1": We1, "l2": We2, "l3": We3, "l4": We4,
          "d1": Wd1, "d2": Wd2, "d3": Wd3, "d4": Wd4}
    Bs = {"l1": be1, "l2": be2, "l3": be3, "l4": be4,
          "d1": bd1, "d2": bd2, "d3": bd3, "d4": bd4}

    in_maps = []
    for c in range(NCORES):
        m = {"xa": xa, "wp": wpa, "bp_rep": bp_rep}
        for name in Ws:
            m[f"w_{name}"] = _prep(np.asarray(Ws[name][c], np.float32), CFG[name])
            m[f"b_{name}"] = np.ascontiguousarray(Bs[name][c], np.float32)
        in_maps.append(m)

    res = run_bass_kernel_spmd(nc, in_maps, list(range(NCORES)))
    results = res.results

    # ---- host-side unshard + combine ----
    xrT_all = np.stack([results[c]["xrT"] for c in range(NCORES)])   # [C, D, B]
    zT_all = np.stack([results[c]["zT"] for c in range(NCORES)])     # [C, L, B]
    errs = np.stack([results[c]["err"][0] for c in range(NCORES)], axis=1)  # [B, C]
    packed = results[0]["p_out"]                                     # [128, B*C/128]
    p = packed.reshape(128, B // 128, C).transpose(1, 0, 2).reshape(B, C)

    z = np.ascontiguousarray(zT_all.transpose(2, 0, 1))              # [B, C, L]

    pd = p.astype(np.float64)
    recon_x = np.zeros((B, D), dtype=np.float64)
    for c in range(NCORES):
        recon_x += xrT_all[c].T.astype(np.float64) * pd[:, c:c + 1]

    errs64 = errs.astype(np.float64)
    recon_loss = (pd * errs64).sum(axis=-1).mean()
    # softmax(-errs) computed stably:
    m_ = (-errs64).max(axis=-1, keepdims=True)
    q = np.exp(-errs64 - m_)
    q /= q.sum(axis=-1, keepdims=True)
    trans_loss = -(q * np.log(pd + 1e-8)).sum(axis=-1).mean()
    mean_p = pd.mean(axis=0)
    nondom_loss = ((mean_p - 1.0 / C) ** 2).sum()
    total_loss = recon_loss + trans_loss + nondom_loss
    xd = x.astype(np.float64)
    mse_loss = ((recon_x - xd) ** 2).mean()

    return (recon_x.astype(np.float32),
            z.astype(np.float32),
            p.astype(np.float32),
            np.float32(total_loss),
            np.float32(recon_loss),
            np.float32(nondom_loss),
            np.float32(trans_loss),
            np.float32(mse_loss))


# revision 34
# speedup vs baseline: 1.0348x; 1.0303x over previous
"""ChartingAE Trainium2 kernel — expert-parallel across 8 NeuronCores.

Sharding: one chart per core (C=8 == n_cores). Each core holds its chart's
encoder/decoder weights resident in SBUF and streams all B=4096 rows through
them in feature-major layout ([feature, batch]), so every layer's matmul
contraction dim is the partition dim with no transposes between layers.

Per-core outputs: raw (unweighted) reconstruction xr^T, latent z^T, per-chart
recon error, and the sparsemax chart probabilities p (computed redundantly on
every core; core 0's copy is used). The p-weighted combine across charts — the
unshard step for expert-parallel — plus the tiny [B,C] loss reductions run on
the host in fp64.

Matmul dtype per layer is configurable: fp16 (1 PE pass, ~2.4e-4 rel) or
fp32r (2 PE passes, ~1.2e-4 rel). ELU is computed branch-free as
  elu(u) = min(exp(u) - 1, max(u, 0))
(for u<=0 the min picks exp(u)-1<=0; for u>0 it picks u since exp(u)-1>u),
costing 1 ACT + 2 DVE ops per tile.
"""

import numpy as np

B, D, LDIM, H, C = 4096, 512, 64, 1024, 8
NCORES = 8
NTILE = 512            # batch tile (moving free dim)
NB = B // NTILE        # 8 batch tiles

# per-layer matmul dtype: "fp16" or "fp32r"
CFG = {
    "l1": "fp16", "l2": "fp16", "l3": "fp16", "l4": "fp16",
    "d1": "fp16", "d2": "fp16", "d3": "fp16", "d4": "fp16",
    "logits": "fp16",
}

LAYERS = [
    # name, in_feat, out_feat, elu
    ("l1", D, H, True),
    ("l2", H, H, True),
    ("l3", H, H, True),
    ("l4", H, LDIM, False),   # -> z
    ("d1", LDIM, H, True),
    ("d2", H, H, True),
    ("d3", H, H, True),
    ("d4", H, D, False),      # -> xr
]

_compiled = {}


def _round_fp32r(a: np.ndarray) -> np.ndarray:
    u = np.ascontiguousarray(a, dtype=np.float32).view(np.uint32).astype(np.uint64)
    lsb = (u >> 12) & 1
    u = (((u + 0x7FF + lsb) >> 12) << 12).astype(np.uint32)
    return u.view(np.float32)


def _prep(a: np.ndarray, kind: str) -> np.ndarray:
    if kind == "fp16":
        return np.ascontiguousarray(a, dtype=np.float16)
    return _round_fp32r(np.ascontiguousarray(a, dtype=np.float32))


def _build():
    """Build + compile the SPMD bass program (identical on all 8 cores)."""
    import concourse.bacc as bacc
    import concourse.tile as tile
    import concourse.mybir as mybir

    dt = mybir.dt
    AF = mybir.ActivationFunctionType
    ALU = mybir.AluOpType

    def bdt(kind):
        return dt.float16 if kind == "fp16" else dt.float32r

    nc = bacc.Bacc("TRN2", target_bir_lowering=False, debug=False)

    # ---- dram parameters ----
    xa = nc.declare_dram_parameter("xa", [D, B], bdt(CFG["l1"]), isOutput=False)
    wps = {}
    bps = {}
    for name, fin, fout, _ in LAYERS:
        wps[name] = nc.declare_dram_parameter(f"w_{name}", [fin, fout], bdt(CFG[name]), isOutput=False)
        bps[name] = nc.declare_dram_parameter(f"b_{name}", [fout], dt.float32, isOutput=False)
    wp = nc.declare_dram_parameter("wp", [D, C], bdt(CFG["logits"]), isOutput=False)
    bp_rep = nc.declare_dram_parameter("bp_rep", [128, C], dt.float32, isOutput=False)

    xrT = nc.declare_dram_parameter("xrT", [D, B], dt.float32, isOutput=True)
    zT = nc.declare_dram_parameter("zT", [LDIM, B], dt.float32, isOutput=True)
    err = nc.declare_dram_parameter("err", [1, B], dt.float32, isOutput=True)
    p_out = nc.declare_dram_parameter("p_out", [128, B * C // 128], dt.float32, isOutput=True)

    with tile.TileContext(nc) as tc:
        import contextlib
        with contextlib.ExitStack() as ctx:
            wpool = ctx.enter_context(tc.tile_pool(name="weights", bufs=1))
            cpool = ctx.enter_context(tc.tile_pool(name="consts", bufs=1))
            xpool = ctx.enter_context(tc.tile_pool(name="x", bufs=3))
            h2b = ctx.enter_context(tc.tile_pool(name="h2b", bufs=36))
            h4b = ctx.enter_context(tc.tile_pool(name="h4b", bufs=12))
            tpool = ctx.enter_context(tc.tile_pool(name="temps", bufs=4))
            opool = ctx.enter_context(tc.tile_pool(name="outs", bufs=6))
            spool = ctx.enter_context(tc.tile_pool(name="small", bufs=3))
            ppool = ctx.enter_context(tc.tile_pool(name="pwork", bufs=1))
            ps = ctx.enter_context(tc.tile_pool(name="ps", bufs=6, space="PSUM"))
            ps1 = ctx.enter_context(tc.tile_pool(name="ps1", bufs=1, space="PSUM"))
            ps2 = ctx.enter_context(tc.tile_pool(name="ps2", bufs=1, space="PSUM"))

            # ---- weights: l1 + logits first so batch-0 compute starts ASAP;
            # the rest are DMA'd after batch-0 L1 is emitted ----
            wt = {}   # name -> list of k-tiles [128, fout]
            bt = {}   # name -> bias tile [128, mtiles]

            def load_w(name):
                fin, fout = dict((n, (fi, fo)) for n, fi, fo, _ in LAYERS)[name]
                kt = (fin + 127) // 128
                tiles = []
                for k in range(kt):
                    kp = min(128, fin - k * 128)
                    t = wpool.tile([kp, fout], bdt(CFG[name]), tag=f"w_{name}_{k}")
                    nc.sync.dma_start(t[:], wps[name][k * 128:k * 128 + kp, :])
                    tiles.append(t)
                wt[name] = tiles

            for name, fin, fout, _ in LAYERS:
                mt = (fout + 127) // 128
                bb = cpool.tile([min(128, fout), mt], dt.float32, tag=f"b_{name}")
                nc.sync.dma_start(
                    bb[:], bps[name][:].rearrange("(m p) -> p m", m=mt))
                bt[name] = bb
            load_w("l1")
            wpt = []
            for k in range(D // 128):
                t = wpool.tile([128, C], bdt(CFG["logits"]), tag=f"wp_{k}")
                nc.sync.dma_start(t[:], wp[k * 128:(k + 1) * 128, :])
                wpt.append(t)
            bpr = cpool.tile([128, C], dt.float32, tag="bp")
            nc.sync.dma_start(bpr[:], bp_rep[:, :])
            ones16 = cpool.tile([128, 1], dt.float16, tag="ones")
            nc.vector.memset(ones16[:], 1.0)
            # sparsemax workspace [128, NB*4*C]
            Lp = ppool.tile([128, B * C // 128], dt.float32, tag="Lp")

            act_dt = bdt(CFG["l1"])

            def act_pool(kind):
                return h2b if kind == "fp16" else h4b

            # ---- software-pipelined emission: Tile schedules each engine in
            # program order, so batch-tile t+1's matmuls can only fill tile
            # t's layer-boundary bubbles if we interleave them statically.
            # Chunk (t, c) = layer c of batch-tile t, emitted by priority
            # t*OFFS + c so two tiles are in flight, offset by OFFS layers.
            OFFS = 4
            state = {}   # nb -> dict(xk=..., cur=...)

            def emit_logits(nb, j):
                # logitsT[b_lo, c] = sum_k x[k, b].T @ wp[k, c] — x slice is
                # the stationary operand, so logits come out batch-major and
                # no transposes are needed
                xkn = state[nb]["xk"]
                lt = ps2.tile([128, C], dt.float32, tag="lps")
                for k in range(D // 128):
                    nc.tensor.matmul(
                        lt[:], xkn[k][:, j * 128:(j + 1) * 128], wpt[k][:],
                        start=(k == 0), stop=(k == D // 128 - 1))
                col = (nb * (NTILE // 128) + j) * C
                nc.vector.tensor_copy(Lp[:, col:col + C], lt[:])

            def emit_x(nb):
                bs = nb * NTILE
                xk = []
                for k in range(D // 128):
                    t = xpool.tile([128, NTILE], act_dt, tag=f"x{k}")
                    nc.sync.dma_start(t[:], xa[k * 128:(k + 1) * 128, bs:bs + NTILE])
                    xk.append(t)
                state[nb] = {"xk": xk, "acts": {-1: xk}}

            def emit_mtile(nb, li, m):
                bs = nb * NTILE
                name, fin, fout, elu = LAYERS[li]
                st = state[nb]
                cur = st["acts"][li - 1]
                xk = st["xk"]
                kt = (fin + 127) // 128
                mt = (fout + 127) // 128
                nxt = st["acts"].setdefault(li, [])
                mp = min(128, fout - m * 128)
                pt = ps.tile([mp, NTILE], dt.float32, tag="ps")
                for k in range(kt):
                    nc.tensor.matmul(
                        pt[:], wt[name][k][:, m * 128:m * 128 + mp], cur[k][:],
                        start=(k == 0), stop=(k == kt - 1))
                bcol = bt[name][0:mp, m:m + 1]
                if elu:
                    # elu(u) = min(exp(u)-1, max(u,0)), u = y + b
                    # engine split keeps ACT/DVE/GpSimd all below PE
                    E = tpool.tile([mp, NTILE], dt.float32, tag="E")
                    nc.scalar.activation(E[:], pt[:], AF.Exp, bias=bcol)
                    r = tpool.tile([mp, NTILE], dt.float32, tag="r")
                    if m % 8 < 3:
                        nc.scalar.activation(r[:], pt[:], AF.Relu, bias=bcol)
                    else:
                        nc.vector.tensor_scalar(r[:], pt[:], bcol, 0.0, ALU.add, ALU.max)
                    hname = "h2" if CFG_next_dtype(name) == "fp16" else "h4"
                    hout = act_pool(CFG_next_dtype(name)).tile(
                        [mp, NTILE], bdt(CFG_next_dtype(name)), tag=hname)
                    nc.vector.scalar_tensor_tensor(
                        hout[:], E[:], 1.0, r[:], ALU.subtract, ALU.min)
                    nxt.append(hout)
                elif name == "l4":
                    # z: export fp32 + feed d1
                    z_s = opool.tile([mp, NTILE], dt.float32, tag="z_s")
                    nc.scalar.activation(z_s[:], pt[:], AF.Identity, bias=bcol)
                    nc.sync.dma_start(zT[:, bs:bs + NTILE], z_s[:])
                    zr = act_pool(CFG_next_dtype(name)).tile(
                        [mp, NTILE], bdt(CFG_next_dtype(name)),
                        tag="h2" if CFG_next_dtype(name) == "fp16" else "h4")
                    nc.scalar.activation(zr[:], pt[:], AF.Identity, bias=bcol)
                    nxt.append(zr)
                else:
                    # d4: xr m-tile -> export + error
                    xr = opool.tile([mp, NTILE], dt.float32, tag="xr")
                    nc.scalar.activation(xr[:], pt[:], AF.Identity, bias=bcol)
                    nc.sync.dma_start(
                        xrT[m * 128:m * 128 + mp, bs:bs + NTILE], xr[:])
                    diff = tpool.tile([mp, NTILE], dt.float32, tag="diff")
                    nc.gpsimd.tensor_tensor(diff[:], xr[:], xk[m][:], ALU.subtract)
                    sq = tpool.tile([mp, NTILE], dt.float16, tag="sq")
                    nc.gpsimd.tensor_tensor(sq[:], diff[:], diff[:], ALU.mult)
                    eps = ps1.tile([1, NTILE], dt.float32, tag="eps")
                    nc.tensor.matmul(eps[:], ones16[:], sq[:],
                                     start=(m == 0), stop=(m == mt - 1))
                    if m == mt - 1:
                        err_s = spool.tile([1, NTILE], dt.float32, tag="err_s")
                        nc.vector.tensor_copy(err_s[:], eps[:])
                        nc.sync.dma_start(err[:, bs:bs + NTILE], err_s[:])
                    nxt.append(xr)
                    # interleave the next tile's logits blocks between the
                    # d4 m-tiles: keeps 213ns streams around the LDW-heavy
                    # little logits matmuls so HAM stays warm
                    if nb + 1 in state:
                        emit_logits(nb + 1, m)
                if nb == 0 and name == "l1" and m == mt - 1:
                    for nm in ("l2", "l3", "l4", "d1", "d2", "d3", "d4"):
                        load_w(nm)
                if name == "l4" and nb == 0:
                    # tile 0 has no predecessor d4 chunk to host its logits
                    for j in range(NTILE // 128):
                        emit_logits(0, j)

            # Chunk (t, c) = layer c of batch-tile t at band t*OFFS+PRIO[c]
            # (decoder at half-priorities so it interleaves with the next
            # tile's encoder), and each chunk's m-tiles spread over a 0.9-wide
            # band so consumer-bound chunks (L1/D1: cheap on PE, expensive on
            # ACT/DVE) interleave at m-tile granularity with neighbors instead
            # of stalling the in-order PE on PSUM-slot recycling.
            PRIO = [0, 1, 2, 3, 4.5, 5.5, 6.5, 7.5]
            tasks = []
            for t in range(NB):
                tasks.append((t * OFFS + PRIO[0] - 0.3, t, -1, 0))
                for c, (nm, fin, fout, _) in enumerate(LAYERS):
                    mt_ = (fout + 127) // 128
                    for m in range(mt_):
                        tasks.append((t * OFFS + PRIO[c] + 0.9 * m / mt_, t, c, m))
            for _, t_, c_, m_ in sorted(tasks):
                if c_ == -1:
                    emit_x(t_)
                else:
                    emit_mtile(t_, c_, m_)

            # ---- sparsemax on Lp [128, NB*4, C] ----
            F = B * C // 128
            G = F // C          # groups of C along free dim
            z3 = Lp[:].rearrange("p (g c) -> p g c", c=C)
            nc.vector.tensor_tensor(
                z3, z3, bpr[:, None, :].broadcast_to([128, G, C]), ALU.add)
            cnt = ppool.tile([128, F], dt.float32, tag="cnt")
            nc.vector.memset(cnt[:], 1.0)
            Tt = ppool.tile([128, F], dt.float32, tag="Tt")
            nc.vector.tensor_copy(Tt[:], Lp[:])
            c3 = cnt[:].rearrange("p (g c) -> p g c", c=C)
            T3 = Tt[:].rearrange("p (g c) -> p g c", c=C)
            ge = ppool.tile([128, F], dt.float32, tag="ge")
            gz = ppool.tile([128, F], dt.float32, tag="gz")
            ge3 = ge[:].rearrange("p (g c) -> p g c", c=C)
            gz3 = gz[:].rearrange("p (g c) -> p g c", c=C)
            for s in range(1, C):
                w = C - s
                # region 1: i in [0,w) vs j=i+s ; region 2 (wrap): i in [w,C) vs j=i-w
                nc.vector.tensor_tensor(ge3[:, :, 0:w], z3[:, :, s:C], z3[:, :, 0:w], ALU.is_ge)
                nc.vector.tensor_tensor(ge3[:, :, w:C], z3[:, :, 0:s], z3[:, :, w:C], ALU.is_ge)
                nc.vector.tensor_tensor(gz3[:, :, 0:w], ge3[:, :, 0:w], z3[:, :, s:C], ALU.mult)
                nc.vector.tensor_tensor(gz3[:, :, w:C], ge3[:, :, w:C], z3[:, :, 0:s], ALU.mult)
                nc.vector.tensor_tensor(cnt[:], cnt[:], ge[:], ALU.add)
                nc.vector.tensor_tensor(Tt[:], Tt[:], gz[:], ALU.add)
            # in_sup = (1 + cnt*z) > T
            u = ppool.tile([128, F], dt.float32, tag="u")
            nc.vector.tensor_tensor(u[:], cnt[:], Lp[:], ALU.mult)
            nc.vector.tensor_scalar(u[:], u[:], 1.0, None, ALU.add)
            sup = ppool.tile([128, F], dt.float32, tag="sup")
            nc.vector.tensor_tensor(sup[:], u[:], Tt[:], ALU.is_gt)
            sup3 = sup[:].rearrange("p (g c) -> p g c", c=C)
            kz = ppool.tile([128, G], dt.float32, tag="kz")
            nc.vector.tensor_reduce(kz[:], sup3, mybir.AxisListType.X, ALU.add)
            zs = ppool.tile([128, F], dt.float32, tag="zs")
            nc.vector.tensor_tensor(zs[:], Lp[:], sup[:], ALU.mult)
            Ss = ppool.tile([128, G], dt.float32, tag="Ss")
            nc.vector.tensor_reduce(Ss[:], zs[:].rearrange("p (g c) -> p g c", c=C), mybir.AxisListType.X, ALU.add)
            nc.vector.tensor_scalar(Ss[:], Ss[:], 1.0, None, ALU.subtract)
            rk = ppool.tile([128, G], dt.float32, tag="rk")
            nc.vector.reciprocal(rk[:], kz[:])
            tau = ppool.tile([128, G], dt.float32, tag="tau")
            nc.vector.tensor_tensor(tau[:], Ss[:], rk[:], ALU.mult)
            pfin = ppool.tile([128, F], dt.float32, tag="pfin")
            tau_b = tau[:, :, None].broadcast_to([128, G, C])
            nc.vector.tensor_tensor(pfin[:].rearrange("p (g c) -> p g c", c=C), z3, tau_b, ALU.subtract)
            nc.vector.tensor_scalar(pfin[:], pfin[:], 0.0, None, ALU.max)
            nc.sync.dma_start(p_out[:, :], pfin[:])

    nc.compile()
    return nc


def CFG_next_dtype(name):
    nxt = {"l1": "l2", "l2": "l3", "l3": "l4", "l4": "d1",
           "d1": "d2", "d2": "d3", "d3": "d4", "d4": None}[name]
    return CFG[nxt] if nxt else "fp32"


def kernel(x, We1, be1, We2, be2, We3, be3, We4, be4,
           Wd1, bd1, Wd2, bd2, Wd3, bd3, Wd4, bd4, Wp, bp):
    from concourse.bass_utils import run_bass_kernel_spmd

    if "nc" not in _compiled:
        _compiled["nc"] = _build()
    nc = _compiled["nc"]

    x = np.asarray(x, dtype=np.float32)
    xT = np.ascontiguousarray(x.T)
    xa = _prep(xT, CFG["l1"])
    wpa = _prep(np.asarray(Wp, np.float32), CFG["logits"])
    bp_rep = np.tile(np.asarray(bp, np.float32)[None, :], (128, 1))

    Ws = {"l1": We1, "l2": We2, "l3": We3, "l4": We4,
          "d1": Wd1, "d2": Wd2, "d3": Wd3, "d4": Wd4}
    Bs = {"l1": be1, "l2": be2, "l3": be3, "l4": be4,
          "d1": bd1, "d2": bd2, "d3": bd3, "d4": bd4}

    in_maps = []
    for c in range(NCORES):
        m = {"xa": xa, "wp": wpa, "bp_rep": bp_rep}
        for name in Ws:
            m[f"w_{name}"] = _prep(np.asarray(Ws[name][c], np.float32), CFG[name])
            m[f"b_{name}"] = np.ascontiguousarray(Bs[name][c], np.float32)
        in_maps.append(m)

    res = run_bass_kernel_spmd(nc, in_maps, list(range(NCORES)))
    results = res.results

    # ---- host-side unshard + combine ----
    xrT_all = np.stack([results[c]["xrT"] for c in range(NCORES)])   # [C, D, B]
    zT_all = np.stack([results[c]["zT"] for c in range(NCORES)])     # [C, L, B]
    errs = np.stack([results[c]["err"][0] for c in range(NCORES)], axis=1)  # [B, C]
    packed = results[0]["p_out"]                                     # [128, B*C/128]
    p = packed.reshape(128, B // 128, C).transpose(1, 0, 2).reshape(B, C)

    z = np.ascontiguousarray(zT_all.transpose(2, 0, 1))              # [B, C, L]

    pd = p.astype(np.float64)
    recon_x = np.zeros((B, D), dtype=np.float64)
    for c in range(NCORES):
        recon_x += xrT_all[c].T.astype(np.float64) * pd[:, c:c + 1]

    errs64 = errs.astype(np.float64)
    recon_loss = (pd * errs64).sum(axis=-1).mean()
    # softmax(-errs) computed stably:
    m_ = (-errs64).max(axis=-1, keepdims=True)
    q = np.exp(-errs64 - m_)
    q /= q.sum(axis=-1, keepdims=True)
    trans_loss = -(q * np.log(pd + 1e-8)).sum(axis=-1).mean()
    mean_p = pd.mean(axis=0)
    nondom_loss = ((mean_p - 1.0 / C) ** 2).sum()
    total_loss = recon_loss + trans_loss + nondom_loss
    xd = x.astype(np.float64)
    mse_loss = ((recon_x - xd) ** 2).mean()

    return (recon_x.astype(np.float32),
            z.astype(np.float32),
            p.astype(np.float32),
            np.float32(total_loss),
            np.float32(recon_loss),
            np.float32(nondom_loss),
            np.float32(trans_loss),
            np.float32(mse_loss))
